# revision 32
# baseline (speedup 1.0000x reference)
"""Multi-head causal attention (B=4, T=2048, D=1024, H=16) on 8 TRN2 NeuronCores.

Sharding: 8 cores = 4 batches x 2 head-halves. Core c handles batch c//2 and
heads [ (c%2)*8, (c%2)*8+8 ).  Each core computes its half of the attention
output and its partial output projection; the host sums the two partial
projections per batch (outputs are bf16; host sums in fp32).

Per-core device kernel (matmul inputs bf16, fp32 PSUM accumulation):
  - Inputs are host-packed so every DMA moves >=2KB contiguous per-partition
    lines (weights 8KB, X in four 1MB t-slabs), spread over 4 trigger queues.
  - Attention is a single software-pipelined stream of 128-wide k-tiles per
    head-pair: S^T tile via row-group-packed K=64 matmul pairs -> exp (ACT)
    -> staircase mask mul (DVE) -> PV accumulation (PSUM, 65 rows: 64 out +
    softmax denominator l from a ones-column in V).  S^T of tile i+1 is
    emitted BEFORE PV of tile i so the in-order TensorE queue never waits on
    the exp of the tile it just produced; this also dissolves the
    qb-boundary pipeline-priming bubbles of the blocked formulation.
  - Projection work (Q/K/V of later pairs, output projections of finished
    query blocks) is queued as single-accumulator 8-matmul chains and
    drained into the attention stream by a ledger that tracks emitted
    TensorE-vs-ACT nanoseconds, so TensorE always has independent work
    wherever ACT paces the exp stream.  Chains carry deadlines (first
    consumer tile); pair 3 reserves its own unused-until-later Q chains as
    the only fill work available during its first (largest) query block.
  - Normalize: l row -> gpsimd copy -> DVE recip (bf16) -> gpsimd partition
    broadcast -> bf16 DVE mul (2x DVE mode).  Deferred into the next block
    so it never head-of-line blocks PV matmuls in the DVE FIFO.
  - Tail: the final 4 output-projection tb groups borrow PSUM slots from the
    (now idle) st/ot pools so 6 accumulators pipeline without bank-reuse
    stalls; evacuations alternate DVE/ACT and output DMAs round-robin over
    3 trigger queues.

No running softmax max is needed: X ~ N(0,1) with 1/sqrt(D)-scaled weights
gives |S/8| < ~10, so exp stays comfortably inside the fp32 range.
"""

import numpy as np
import ml_dtypes

import concourse.bass as bass
import concourse.mybir as mybir
import concourse.tile as tile
from concourse import bacc
from concourse import bass_utils

BF16 = mybir.dt.bfloat16
F32 = mybir.dt.float32
AF = mybir.ActivationFunctionType

B, T, D = 4, 2048, 1024
H, DK = 16, 64
HALF = 512            # channels per core (8 heads)
KB = D // 128         # 8 contraction blocks for projections
TB = T // 128         # 16 t/k blocks of 128
QB = T // 512         # 4 query blocks of 512
NPAIR = 4             # head pairs per core (2 heads = 128 channels)
SCALE = float(DK) ** -0.5

N_CORES = 8

_PROG = None  # compiled program cache


def _build_program():
    nc = bacc.Bacc("TRN2", target_bir_lowering=False, debug=False)

    # host-packed layouts: per-partition lines are large and contiguous
    xt_d = nc.dram_tensor("xt", [4, 128, KB, 512], BF16, kind="ExternalInput")
    wqt_d = nc.dram_tensor("wqt", [128, KB, HALF], BF16, kind="ExternalInput")
    wkt_d = nc.dram_tensor("wkt", [128, KB, HALF], BF16, kind="ExternalInput")
    wvt_d = nc.dram_tensor("wvt", [128, KB, HALF], BF16, kind="ExternalInput")
    wot_d = nc.dram_tensor("wot", [128, 4, D], BF16, kind="ExternalInput")
    mask_d = nc.dram_tensor("mask", [128, 2, 128], BF16, kind="ExternalInput")
    out_d = nc.dram_tensor("out", [TB, 128, D], BF16, kind="ExternalOutput")

    with tile.TileContext(nc) as tc:
        with (
            tc.tile_pool(name="const", bufs=1) as const,
            tc.tile_pool(name="sb_pt", bufs=4) as sb_pt,
            tc.tile_pool(name="sb_otu", bufs=6) as sb_otu,
            tc.tile_pool(name="sb_lr", bufs=8) as sb_lr,
            tc.tile_pool(name="sb_rbr", bufs=4) as sb_rbr,
            tc.tile_pool(name="sb_sc", bufs=4) as sb_sc,
            tc.tile_pool(name="sb_out", bufs=6) as sb_out,
            tc.tile_pool(name="ps_st", bufs=2, space="PSUM") as ps_st,
            tc.tile_pool(name="ps_ot", bufs=2, space="PSUM") as ps_ot,
            tc.tile_pool(name="ps_acc", bufs=2, space="PSUM") as ps_acc,
        ):
            # four separate t-slab tiles: each is written by ONE whole-tile
            # DMA, so a consumer's dependency is exactly the slab it reads
            xt_sbs = [
                const.tile([128, KB, 512], BF16, tag=f"xt{q}", name=f"xt{q}")
                for q in range(4)
            ]
            wqt_sb = const.tile([128, KB, HALF], BF16, tag="wqt")
            wkt_sb = const.tile([128, KB, HALF], BF16, tag="wkt")
            wvt_sb = const.tile([128, KB, HALF], BF16, tag="wvt")
            wot_sb = const.tile([128, 4, D], BF16, tag="wot")
            mask_sb = const.tile([128, 2, 128], BF16, tag="mask")
            warm_sb = const.tile([128, 512], BF16, tag="warm")
            qt_sb = const.tile([128, NPAIR, T], BF16, tag="qt")
            kt_sb = const.tile([128, NPAIR, T], BF16, tag="kt")
            vaug_sb = const.tile([128, TB, 8 * 65], BF16, tag="vaug")
            otn_sb = const.tile([128, NPAIR, T], BF16, tag="otn")

            # HAM warm-up with no DMA dependency: matmul a memset tile so the
            # clock gate opens / p-state ramps while the input DMAs are in
            # flight; sized to roughly cover the first projection group's
            # input landing time.
            nc.vector.memset(warm_sb[:], 0.0)
            warm = ps_acc.tile([128, 512], F32, tag="acc")
            NWARM = 38
            for w in range(NWARM):
                # N=256: half the row cost for the same wall-clock span --
                # the point is keeping the clock gate open during the input
                # DMA ramp, not doing work
                nc.tensor.matmul(
                    warm[:, 0:256],
                    warm_sb[:, 0:128],
                    warm_sb[:, 0:256],
                    start=(w == 0),
                    stop=(w == NWARM - 1),
                )

            # ---- input DMAs: need-ordered over the 3 DMA-capable trigger
            # queues; whole xt slabs (1MB, 8KB lines) so a consumer's wait is
            # exactly its slab; weights in 2 chunks each for queue overlap.
            engs = [nc.sync, nc.scalar, nc.gpsimd]
            _n = [0]

            def dma_in(dst, src):
                engs[_n[0] % len(engs)].dma_start(dst, src)
                _n[0] += 1

            def w_chunks(sb, dr):  # weights: 2 chunks of [128, 4, 512]
                for c in range(2):
                    dma_in(sb[:, 4 * c : 4 * c + 4, :], dr.ap()[:, 4 * c : 4 * c + 4, :])

            def x_chunks(q):  # t-slab: 2 chunks of [128, 4, 512] on 2 queues
                for c in range(2):
                    dma_in(
                        xt_sbs[q][:, 4 * c : 4 * c + 4, :],
                        xt_d.ap()[q][:, 4 * c : 4 * c + 4, :],
                    )

            w_chunks(wqt_sb, wqt_d)
            x_chunks(0)
            w_chunks(wkt_sb, wkt_d)
            w_chunks(wvt_sb, wvt_d)
            x_chunks(1)
            x_chunks(2)
            x_chunks(3)
            for c in range(2):  # wot: 2 chunks of [128, 2, 1024]
                dma_in(wot_sb[:, 2 * c : 2 * c + 2, :], wot_d.ap()[:, 2 * c : 2 * c + 2, :])
            nc.sync.dma_start(mask_sb[:], mask_d.ap())
            for h in range(8):  # ones column per head in V_aug
                nc.vector.memset(vaug_sb[:, :, h * 65 + 64 : h * 65 + 65], 1.0)

            # ---- projection chains (single 8-matmul accumulator each) ----
            out_q = [0]  # rotating output DMA queue
            out_engs = [nc.sync, nc.gpsimd]

            def qk_chain(pair, dst_i, nbp, h):
                dst_sb, w_sb = ((qt_sb, wqt_sb), (kt_sb, wkt_sb))[dst_i]
                slab = xt_sbs[nbp * 2 + h]
                tsl = slice(nbp * 1024 + h * 512, nbp * 1024 + (h + 1) * 512)

                def fn():
                    acc = ps_acc.tile([128, 512], F32, tag="acc")
                    for kb in range(KB):
                        nc.tensor.matmul(
                            acc[:],
                            w_sb[:, kb, pair * 128 : (pair + 1) * 128],
                            slab[:, kb, :],
                            start=(kb == 0),
                            stop=(kb == KB - 1),
                        )
                    nc.vector.tensor_copy(dst_sb[:, pair, tsl], acc[:])

                return fn

            def v_chain(tidx):
                slab = xt_sbs[tidx // 4]
                lsl = slice((tidx % 4) * 128, (tidx % 4) * 128 + 128)

                def fn():
                    acc = ps_acc.tile([128, 512], F32, tag="acc")
                    for kb in range(KB):
                        nc.tensor.matmul(
                            acc[:],
                            slab[:, kb, lsl],
                            wvt_sb[:, kb, :],
                            start=(kb == 0),
                            stop=(kb == KB - 1),
                        )
                    nc.vector.tensor_copy(
                        vaug_sb[:, tidx, :].rearrange("p (h c) -> p h c", c=65)[
                            :, :, 0:64
                        ],
                        acc[:].rearrange("p (h c) -> p h c", c=64),
                    )

                return fn

            TAIL_MODE = [False]
            tail_rot = [0]

            def o_chain(tb, half, pool=None, evac=0):
                tsl = slice(tb * 128, (tb + 1) * 128)
                csl = slice(half * 512, (half + 1) * 512)

                def fn():
                    if pool is not None:
                        p = pool
                    elif TAIL_MODE[0]:
                        # post-stream: rotate over all three PSUM pools (the
                        # st/ot slots are idle once attention has finished)
                        p = (ps_acc, ps_acc, ps_st, ps_st, ps_ot, ps_ot)[
                            tail_rot[0] % 6
                        ]
                        tail_rot[0] += 1
                    else:
                        p = ps_acc
                    acc = p.tile([128, 512], F32, tag=p._o_tag)
                    for cb in range(4):
                        nc.tensor.matmul(
                            acc[:],
                            otn_sb[:, cb, tsl],
                            wot_sb[:, cb, csl],
                            start=(cb == 0),
                            stop=(cb == 3),
                        )
                    outc = sb_out.tile([128, 512], BF16, tag="outc")
                    if evac == 1:
                        nc.scalar.copy(outc[:], acc[:])
                    else:
                        nc.vector.tensor_copy(outc[:], acc[:])
                    out_engs[out_q[0] % len(out_engs)].dma_start(
                        out_d.ap()[tb][:, csl], outc[:]
                    )
                    out_q[0] += 1

                return fn

            ps_acc._o_tag = "acc"
            ps_st._o_tag = "st"
            ps_ot._o_tag = "ot"

            # ---- flat attention tile stream ----
            blocks = [(p, qb) for p in range(3) for qb in range(4)]
            blocks += [(3, qb) for qb in (3, 2, 1, 0)]
            tiles = []
            bstart = {}
            for bi, (p, qb) in enumerate(blocks):
                bstart[bi] = len(tiles)
                for j in range(4 * qb + 4):
                    tiles.append((bi, p, qb, j))
            NT = len(tiles)

            def tile_geom(qb, j):
                d = j - 4 * qb
                lo = 128 * d if d >= 1 else 0
                return d, lo, 512 - lo

            # ledger: ns of TensorE work vs ACT work emitted so far
            led = {"T": 0.0, "A": 0.0}
            LEAD = 3600.0

            # fill queue: (deadline_tile_idx, cost_ns, fn)
            from collections import deque

            fill_q = deque()
            QK_COST, O_COST = 1720.0, 880.0

            def drain_one():
                _, cost, fn = fill_q.popleft()
                fn()
                led["T"] += cost

            def drain_forced(i):
                while fill_q and fill_q[0][0] <= i:
                    drain_one()

            def drain_voluntary():
                # at most one chain per tile: smooth injection keeps the
                # exp pipeline (bounded by st double-buffering) fed
                if fill_q and led["T"] < led["A"] + LEAD:
                    drain_one()

            # upfront: exactly what block (p0,qb0) needs beyond the queue
            qk_chain(0, 0, 0, 0)()  # Q pair0 t 0:512
            qk_chain(0, 1, 0, 0)()  # K pair0 k 0:512
            for tidx in range(4):
                v_chain(tidx)()

            # static fills with deadlines (monotone)
            def bs(p, qb):
                return bstart[blocks.index((p, qb))]

            # Deadlines are the stream index of each chain's FIRST consumer
            # tile (Q h-slab -> first S of its query block; K h-slab -> first
            # S with j in its k range; V t-tile -> its PV tile), so forced
            # drains spread instead of bursting at block starts.
            ddl = []
            ddl.append((bs(0, 1), qk_chain(0, 0, 0, 1)))          # Q q512:1024
            ddl.append((bs(0, 1) + 4, qk_chain(0, 1, 0, 1)))      # K k512:1024
            for tidx in range(4, 8):
                ddl.append((bs(0, 1) + tidx, v_chain(tidx)))
            ddl.append((bs(0, 2), qk_chain(0, 0, 1, 0)))
            ddl.append((bs(0, 2) + 8, qk_chain(0, 1, 1, 0)))
            for tidx in range(8, 12):
                ddl.append((bs(0, 2) + tidx, v_chain(tidx)))
            ddl.append((bs(0, 3), qk_chain(0, 0, 1, 1)))
            ddl.append((bs(0, 3) + 12, qk_chain(0, 1, 1, 1)))
            for tidx in range(12, 16):
                ddl.append((bs(0, 3) + tidx, v_chain(tidx)))
            for p in (1, 2):
                ddl.append((bs(p, 0), qk_chain(p, 0, 0, 0)))
                ddl.append((bs(p, 0), qk_chain(p, 1, 0, 0)))
                ddl.append((bs(p, 1), qk_chain(p, 0, 0, 1)))
                ddl.append((bs(p, 1) + 4, qk_chain(p, 1, 0, 1)))
                ddl.append((bs(p, 2), qk_chain(p, 0, 1, 0)))
                ddl.append((bs(p, 2) + 8, qk_chain(p, 1, 1, 0)))
                ddl.append((bs(p, 3), qk_chain(p, 0, 1, 1)))
                ddl.append((bs(p, 3) + 12, qk_chain(p, 1, 1, 1)))
            # pair 3 (descending walk, qb3 first): K h-slabs staggered by
            # first consumer j; the three late-query Q chains are reserved as
            # pair3's only fill work during its largest blocks.
            ddl.append((bs(3, 3), qk_chain(3, 1, 0, 0)))
            ddl.append((bs(3, 3), qk_chain(3, 0, 1, 1)))          # Q q1536:2048
            ddl.append((bs(3, 3) + 4, qk_chain(3, 1, 0, 1)))
            ddl.append((bs(3, 3) + 8, qk_chain(3, 1, 1, 0)))
            ddl.append((bs(3, 3) + 12, qk_chain(3, 1, 1, 1)))
            ddl.append((bs(3, 2), qk_chain(3, 0, 1, 0)))          # q 1024:1536
            ddl.append((bs(3, 1), qk_chain(3, 0, 0, 1)))          # q  512:1024
            ddl.append((bs(3, 0), qk_chain(3, 0, 0, 0)))          # q    0:512
            ddl.sort(key=lambda e: e[0])
            for dd, fn in ddl:
                fill_q.append((dd, QK_COST, fn))

            # ---- per-tile emitters ----
            tile_pt = {}
            block_ot = {}

            def emit_S(i):
                bi, p, qb, j = tiles[i]
                d, lo, N = tile_geom(qb, j)
                vq = slice(qb * 512 + lo, (qb + 1) * 512)
                jsl = slice(j * 128, (j + 1) * 128)
                st = ps_st.tile([128, 1024], F32, tag="st")
                st3 = st[:].rearrange("p (h q) -> p h q", h=2)
                nc.tensor.matmul(
                    st3[:, 0, lo:512], kt_sb[0:64, p, jsl], qt_sb[0:64, p, vq]
                )
                nc.tensor.matmul(
                    st3[:, 1, lo:512], kt_sb[64:128, p, jsl], qt_sb[64:128, p, vq]
                )
                led["T"] += 0.78 * N
                return st3

            def emit_E(i, st3):
                bi, p, qb, j = tiles[i]
                d, lo, N = tile_geom(qb, j)
                pt = sb_pt.tile([128, 1024], BF16, tag="pt")
                pt3 = pt[:].rearrange("p (h q) -> p h q", h=2)
                nc.scalar.activation(
                    pt3[:, :, lo:512], st3[:, :, lo:512], AF.Exp, scale=SCALE
                )
                if d >= 0:
                    nc.vector.tensor_mul(
                        pt3[:, :, lo : lo + 128],
                        pt3[:, :, lo : lo + 128],
                        mask_sb[:],
                    )
                led["A"] += 240.0 + 1.71 * N
                tile_pt[i] = pt3

            def emit_PV(i):
                bi, p, qb, j = tiles[i]
                d, lo, N = tile_geom(qb, j)
                pt3 = tile_pt.pop(i)
                first = j == 0
                last = i + 1 >= NT or tiles[i + 1][0] != bi
                if first:
                    ot0 = ps_ot.tile([128, 512], F32, tag="ot")
                    ot1 = ps_ot.tile([128, 512], F32, tag="ot")
                    block_ot[bi] = (ot0, ot1)
                ot0, ot1 = block_ot[bi]
                h0 = 2 * p
                nc.tensor.matmul(
                    ot0[0:65, lo:512],
                    vaug_sb[:, j, h0 * 65 : (h0 + 1) * 65],
                    pt3[:, 0, lo:512],
                    start=first,
                    stop=last,
                )
                nc.tensor.matmul(
                    ot1[0:65, lo:512],
                    vaug_sb[:, j, (h0 + 1) * 65 : (h0 + 2) * 65],
                    pt3[:, 1, lo:512],
                    start=first,
                    stop=last,
                )
                led["T"] += 0.85 * N
                return last

            def part1(bi, tail=False):
                # evacuate O^T and the l row promptly to release the ot banks
                ot0, ot1 = block_ot.pop(bi)
                otus, lrows = [], []
                for hh, ot in ((0, ot0), (1, ot1)):
                    otu = sb_otu.tile([64, 512], BF16, tag="otu")
                    if tail and hh == 1:
                        nc.scalar.copy(otu[:], ot[0:64, :])
                    else:
                        nc.vector.tensor_copy(otu[:], ot[0:64, :])
                    lrow = sb_lr.tile([1, 512], F32, tag="lrow")
                    if hh == 0:
                        nc.scalar.copy(lrow[:], ot[64:65, :])
                    else:
                        nc.vector.tensor_copy(lrow[:], ot[64:65, :])
                    otus.append(otu)
                    lrows.append(lrow)
                return otus, lrows

            def make_part2(bi, otus, lrows, mul_eng=None):
                p, qb = blocks[bi]
                qsl = slice(qb * 512, (qb + 1) * 512)
                eng = mul_eng if mul_eng is not None else nc.vector

                def fn():
                    # both recips first so the muls (which wait on the gpsimd
                    # broadcasts) never head-of-line block the DVE queue;
                    # h1 first: its partition-crossing DMA overlaps h0's mul
                    rbrs = {}
                    for hh in (1, 0):
                        rec = sb_lr.tile([1, 512], F32, tag="rec")
                        nc.vector.reciprocal_approx_fast(rec[:], lrows[hh][:])
                        rbr = sb_rbr.tile([64, 512], F32, tag="rbr")
                        nc.gpsimd.partition_broadcast(rbr[:], rec[0:1, :])
                        rbrs[hh] = rbr
                    sc = sb_sc.tile([64, 512], BF16, tag="sc")
                    eng.tensor_mul(sc[:], otus[1][:], rbrs[1][:])
                    nc.sync.dma_start(otn_sb[64:128, p, qsl], sc[:])
                    eng.tensor_mul(otn_sb[0:64, p, qsl], otus[0][:], rbrs[0][:])

                return fn

            # ---- main pipelined loop ----
            part2_q = []  # (due_iteration, fn, block_idx)

            def maybe_part2(k):
                while part2_q and k >= part2_q[0][0]:
                    due, fn, bi = part2_q.pop(0)
                    fn()
                    p, qb = blocks[bi]
                    if p == 3 and qb > 0:
                        for tb in range(4 * qb, 4 * qb + 4):
                            for half in (0, 1):
                                fill_q.append((NT + 1, O_COST, o_chain(tb, half)))

            def sched_part2(k, bi_prev):
                # Normal: emit the deferred normalize at the 2nd tile of the
                # next block.  For pair-transition blocks (p,3) p<3 the otn
                # output has no consumer until pair 3, and the next block
                # (qb0) is all-diagonal (its mask-muls would sit behind the
                # part2 DVE work): defer clear into (p+1,qb2), whose first
                # diagonal tile is 8 tiles in.  NOTE: gpsimd cannot take the
                # muls -- switching gpsimd between tensor ops and
                # partition_broadcast forces a ~7us library reload.
                p, qb = blocks[bi_prev]
                if qb == 3 and bi_prev + 1 < len(blocks) and blocks[bi_prev + 1][1] == 0:
                    due = bstart[bi_prev + 3] + 2
                else:
                    due = k + 1
                otus, lrows = part1(bi_prev)
                entry = (due, make_part2(bi_prev, otus, lrows), bi_prev)
                part2_q.append(entry)
                part2_q.sort(key=lambda e: e[0])

            for k in range(NT):
                drain_forced(k)
                st3 = emit_S(k)
                emit_E(k, st3)
                maybe_part2(k)
                drain_voluntary()
                if k > 0:
                    last = emit_PV(k - 1)
                    if last:
                        sched_part2(k, tiles[k - 1][0])

            emit_PV(NT - 1)
            bi_last = tiles[NT - 1][0]
            otus, lrows = part1(bi_last, tail=True)
            maybe_part2(NT)  # flush any deferred part2s
            make_part2(bi_last, otus, lrows)()

            # post-stream: remaining out-proj chains (qb1's, then qb0's)
            # rotate over all three PSUM pools so 6 accumulators pipeline;
            # evacuations alternate DVE/ACT.
            TAIL_MODE[0] = True
            while fill_q:
                drain_one()
            n = 0
            for tb in range(4):
                for half in (0, 1):
                    o_chain(tb, half, evac=n % 2)()
                    n += 1

    nc.compile()
    return nc


def _prep_core_inputs(X, W_q, W_k, W_v, W_o, mask_host, c):
    b, half = c // 2, c % 2
    ch = slice(half * HALF, (half + 1) * HALF)
    bf = ml_dtypes.bfloat16
    xtp = np.ascontiguousarray(
        X[b].T.reshape(KB, 128, T).transpose(1, 0, 2)
    )  # [128, KB, T]
    xt = np.ascontiguousarray(
        np.stack([xtp[:, :, q * 512 : (q + 1) * 512] for q in range(4)])
    ).astype(bf)  # [4, 128, KB, 512]
    wqt = np.ascontiguousarray(
        W_q[ch, :].T.reshape(KB, 128, HALF).transpose(1, 0, 2)
    ).astype(bf)
    wkt = np.ascontiguousarray(
        W_k[ch, :].T.reshape(KB, 128, HALF).transpose(1, 0, 2)
    ).astype(bf)
    wvt = np.ascontiguousarray(
        W_v[ch, :].T.reshape(KB, 128, HALF).transpose(1, 0, 2)
    ).astype(bf)
    wot = np.ascontiguousarray(
        W_o[:, ch].T.reshape(4, 128, D).transpose(1, 0, 2)
    ).astype(bf)
    return {
        "xt": xt, "wqt": wqt, "wkt": wkt, "wvt": wvt, "wot": wot,
        "mask": mask_host,
    }


def _make_mask():
    kp = np.arange(128)[:, None]
    qf = np.arange(128)[None, :]
    keep = (qf >= kp).astype(np.float32)
    m = np.zeros((128, 2, 128), np.float32)
    m[:, 0, :] = keep
    m[:, 1, :] = keep
    return m.astype(ml_dtypes.bfloat16)


def kernel(X, W_q, W_k, W_v, W_o):
    global _PROG
    X = np.asarray(X, dtype=np.float32)
    W_q = np.asarray(W_q, dtype=np.float32)
    W_k = np.asarray(W_k, dtype=np.float32)
    W_v = np.asarray(W_v, dtype=np.float32)
    W_o = np.asarray(W_o, dtype=np.float32)

    if _PROG is None:
        _PROG = _build_program()
    nc = _PROG

    mask_host = _make_mask()
    in_maps = [
        _prep_core_inputs(X, W_q, W_k, W_v, W_o, mask_host, c)
        for c in range(N_CORES)
    ]
    res = bass_utils.run_bass_kernel_spmd(nc, in_maps, core_ids=list(range(N_CORES)))

    out = np.empty((B, T, D), np.float32)
    for b in range(B):
        p0 = res.results[2 * b]["out"].reshape(T, D).astype(np.float32)
        p1 = res.results[2 * b + 1]["out"].reshape(T, D).astype(np.float32)
        out[b] = p0 + p1
    return out


# revision 33
# speedup vs baseline: 1.1865x; 1.1865x over previous
"""Multi-head causal attention (B=4, T=2048, D=1024, H=16) on 8 TRN2 NeuronCores.

Sharding: 8 cores = 4 batches x 2 head-halves. Core c handles batch c//2 and
heads [ (c%2)*8, (c%2)*8+8 ).  Each core computes its half of the attention
output and its partial output projection; the host sums the two partial
projections per batch (outputs are bf16; host sums in fp32).

Per-core device kernel (matmul inputs bf16, fp32 PSUM accumulation):
  - Inputs are host-packed so every DMA moves >=2KB contiguous per-partition
    lines (weights 8KB, X in four 1MB t-slabs), spread over 4 trigger queues.
  - Attention is a single software-pipelined stream of 128-wide k-tiles per
    head-pair: S^T tile via row-group-packed K=64 matmul pairs -> exp (ACT)
    -> staircase mask mul (DVE) -> PV accumulation (PSUM, 65 rows: 64 out +
    softmax denominator l from a ones-column in V).  S^T of tile i+1 is
    emitted BEFORE PV of tile i so the in-order TensorE queue never waits on
    the exp of the tile it just produced; this also dissolves the
    qb-boundary pipeline-priming bubbles of the blocked formulation.
  - Projection work (Q/K/V of later pairs, output projections of finished
    query blocks) is queued as single-accumulator 8-matmul chains and
    drained into the attention stream by a ledger that tracks emitted
    TensorE-vs-ACT nanoseconds, so TensorE always has independent work
    wherever ACT paces the exp stream.  Chains carry deadlines (first
    consumer tile); pair 3 reserves its own unused-until-later Q chains as
    the only fill work available during its first (largest) query block.
  - Normalize: l row -> gpsimd copy -> DVE recip (bf16) -> gpsimd partition
    broadcast -> bf16 DVE mul (2x DVE mode).  Deferred into the next block
    so it never head-of-line blocks PV matmuls in the DVE FIFO.
  - Tail: the final 4 output-projection tb groups borrow PSUM slots from the
    (now idle) st/ot pools so 6 accumulators pipeline without bank-reuse
    stalls; evacuations alternate DVE/ACT and output DMAs round-robin over
    3 trigger queues.

No running softmax max is needed: X ~ N(0,1) with 1/sqrt(D)-scaled weights
gives |S/8| < ~10, so exp stays comfortably inside the fp32 range.
"""

import numpy as np
import ml_dtypes

import concourse.bass as bass
import concourse.mybir as mybir
import concourse.tile as tile
from concourse import bacc
from concourse import bass_utils

BF16 = mybir.dt.bfloat16
F32 = mybir.dt.float32
AF = mybir.ActivationFunctionType

B, T, D = 4, 2048, 1024
H, DK = 16, 64
HALF = 512            # channels per core (8 heads)
KB = D // 128         # 8 contraction blocks for projections
TB = T // 128         # 16 t/k blocks of 128
QB = T // 512         # 4 query blocks of 512
NPAIR = 4             # head pairs per core (2 heads = 128 channels)
SCALE = float(DK) ** -0.5

N_CORES = 8

_PROG = None  # compiled program cache


def _build_program():
    nc = bacc.Bacc("TRN2", target_bir_lowering=False, debug=False)

    # host-packed layouts: per-partition lines are large and contiguous
    xt_d = nc.dram_tensor("xt", [4, 128, KB, 512], BF16, kind="ExternalInput")
    wqt_d = nc.dram_tensor("wqt", [128, KB, HALF], BF16, kind="ExternalInput")
    wkt_d = nc.dram_tensor("wkt", [128, KB, HALF], BF16, kind="ExternalInput")
    wvt_d = nc.dram_tensor("wvt", [128, KB, HALF], BF16, kind="ExternalInput")
    wot_d = nc.dram_tensor("wot", [128, 4, D], BF16, kind="ExternalInput")
    mask_d = nc.dram_tensor("mask", [128, 2, 128], BF16, kind="ExternalInput")
    out_d = nc.dram_tensor("out", [TB, 128, D], BF16, kind="ExternalOutput")

    with tile.TileContext(nc) as tc:
        with (
            tc.tile_pool(name="const", bufs=1) as const,
            tc.tile_pool(name="sb_pt", bufs=4) as sb_pt,
            tc.tile_pool(name="sb_otu", bufs=6) as sb_otu,
            tc.tile_pool(name="sb_lr", bufs=8) as sb_lr,
            tc.tile_pool(name="sb_rbr", bufs=4) as sb_rbr,
            tc.tile_pool(name="sb_sc", bufs=4) as sb_sc,
            tc.tile_pool(name="sb_out", bufs=6) as sb_out,
            tc.tile_pool(name="ps_st", bufs=2, space="PSUM") as ps_st,
            tc.tile_pool(name="ps_ot", bufs=2, space="PSUM") as ps_ot,
            tc.tile_pool(name="ps_acc", bufs=2, space="PSUM") as ps_acc,
        ):
            # four separate t-slab tiles: each is written by ONE whole-tile
            # DMA, so a consumer's dependency is exactly the slab it reads
            xt_sbs = [
                const.tile([128, KB, 512], BF16, tag=f"xt{q}", name=f"xt{q}")
                for q in range(4)
            ]
            wqt_sb = const.tile([128, KB, HALF], BF16, tag="wqt")
            wkt_sb = const.tile([128, KB, HALF], BF16, tag="wkt")
            wvt_sb = const.tile([128, KB, HALF], BF16, tag="wvt")
            wot_sb = const.tile([128, 4, D], BF16, tag="wot")
            mask_sb = const.tile([128, 2, 128], BF16, tag="mask")
            warm_sb = const.tile([128, 512], BF16, tag="warm")
            qt_sb = const.tile([128, NPAIR, T], BF16, tag="qt")
            kt_sb = const.tile([128, NPAIR, T], BF16, tag="kt")
            vaug_sb = const.tile([128, TB, 8 * 65], BF16, tag="vaug")
            otn_sb = const.tile([128, NPAIR, T], BF16, tag="otn")

            # HAM warm-up with no DMA dependency: matmul a memset tile so the
            # clock gate opens / p-state ramps while the input DMAs are in
            # flight; sized to roughly cover the first projection group's
            # input landing time.
            nc.vector.memset(warm_sb[:], 0.0)
            warm = ps_acc.tile([128, 512], F32, tag="acc")
            NWARM = 38
            for w in range(NWARM):
                nc.tensor.matmul(
                    warm[:],
                    warm_sb[:, 0:128],
                    warm_sb[:],
                    start=(w == 0),
                    stop=(w == NWARM - 1),
                )

            # ---- input DMAs: need-ordered over the 3 DMA-capable trigger
            # queues; whole xt slabs (1MB, 8KB lines) so a consumer's wait is
            # exactly its slab; weights in 2 chunks each for queue overlap.
            engs = [nc.sync, nc.scalar, nc.gpsimd]
            _n = [0]

            def dma_in(dst, src):
                engs[_n[0] % len(engs)].dma_start(dst, src)
                _n[0] += 1

            def w_chunks(sb, dr):  # weights: 2 chunks of [128, 4, 512]
                for c in range(2):
                    dma_in(sb[:, 4 * c : 4 * c + 4, :], dr.ap()[:, 4 * c : 4 * c + 4, :])

            def x_chunks(q):  # t-slab: 2 chunks of [128, 4, 512] on 2 queues
                for c in range(2):
                    dma_in(
                        xt_sbs[q][:, 4 * c : 4 * c + 4, :],
                        xt_d.ap()[q][:, 4 * c : 4 * c + 4, :],
                    )

            w_chunks(wqt_sb, wqt_d)
            x_chunks(0)
            w_chunks(wkt_sb, wkt_d)
            w_chunks(wvt_sb, wvt_d)
            x_chunks(1)
            x_chunks(2)
            x_chunks(3)
            for c in range(2):  # wot: 2 chunks of [128, 2, 1024]
                dma_in(wot_sb[:, 2 * c : 2 * c + 2, :], wot_d.ap()[:, 2 * c : 2 * c + 2, :])
            nc.sync.dma_start(mask_sb[:], mask_d.ap())
            for h in range(8):  # ones column per head in V_aug
                nc.vector.memset(vaug_sb[:, :, h * 65 + 64 : h * 65 + 65], 1.0)

            # ---- projection chains (single 8-matmul accumulator each) ----
            out_q = [0]  # rotating output DMA queue
            out_engs = [nc.sync, nc.gpsimd]

            def qk_chain(pair, dst_i, nbp, h):
                dst_sb, w_sb = ((qt_sb, wqt_sb), (kt_sb, wkt_sb))[dst_i]
                slab = xt_sbs[nbp * 2 + h]
                tsl = slice(nbp * 1024 + h * 512, nbp * 1024 + (h + 1) * 512)

                def fn():
                    acc = ps_acc.tile([128, 512], F32, tag="acc")
                    for kb in range(KB):
                        nc.tensor.matmul(
                            acc[:],
                            w_sb[:, kb, pair * 128 : (pair + 1) * 128],
                            slab[:, kb, :],
                            start=(kb == 0),
                            stop=(kb == KB - 1),
                        )
                    nc.vector.tensor_copy(dst_sb[:, pair, tsl], acc[:])

                return fn

            def v_chain(tidx):
                slab = xt_sbs[tidx // 4]
                lsl = slice((tidx % 4) * 128, (tidx % 4) * 128 + 128)

                def fn():
                    acc = ps_acc.tile([128, 512], F32, tag="acc")
                    for kb in range(KB):
                        nc.tensor.matmul(
                            acc[:],
                            slab[:, kb, lsl],
                            wvt_sb[:, kb, :],
                            start=(kb == 0),
                            stop=(kb == KB - 1),
                        )
                    nc.vector.tensor_copy(
                        vaug_sb[:, tidx, :].rearrange("p (h c) -> p h c", c=65)[
                            :, :, 0:64
                        ],
                        acc[:].rearrange("p (h c) -> p h c", c=64),
                    )

                return fn

            TAIL_MODE = [False]
            tail_rot = [0]

            def o_chain(tb, half, pool=None, evac=0):
                tsl = slice(tb * 128, (tb + 1) * 128)
                csl = slice(half * 512, (half + 1) * 512)

                def fn():
                    if pool is not None:
                        p = pool
                    elif TAIL_MODE[0]:
                        # post-stream: rotate over all three PSUM pools (the
                        # st/ot slots are idle once attention has finished)
                        p = (ps_acc, ps_acc, ps_st, ps_st, ps_ot, ps_ot)[
                            tail_rot[0] % 6
                        ]
                        tail_rot[0] += 1
                    else:
                        p = ps_acc
                    acc = p.tile([128, 512], F32, tag=p._o_tag)
                    for cb in range(4):
                        nc.tensor.matmul(
                            acc[:],
                            otn_sb[:, cb, tsl],
                            wot_sb[:, cb, csl],
                            start=(cb == 0),
                            stop=(cb == 3),
                        )
                    outc = sb_out.tile([128, 512], BF16, tag="outc")
                    if evac == 1:
                        nc.scalar.copy(outc[:], acc[:])
                    else:
                        nc.vector.tensor_copy(outc[:], acc[:])
                    out_engs[out_q[0] % len(out_engs)].dma_start(
                        out_d.ap()[tb][:, csl], outc[:]
                    )
                    out_q[0] += 1

                return fn

            ps_acc._o_tag = "acc"
            ps_st._o_tag = "st"
            ps_ot._o_tag = "ot"

            # ---- flat attention tile stream ----
            blocks = [(p, qb) for p in range(3) for qb in range(4)]
            blocks += [(3, qb) for qb in (3, 2, 1, 0)]
            tiles = []
            bstart = {}
            for bi, (p, qb) in enumerate(blocks):
                bstart[bi] = len(tiles)
                for j in range(4 * qb + 4):
                    tiles.append((bi, p, qb, j))
            NT = len(tiles)

            def tile_geom(qb, j):
                d = j - 4 * qb
                lo = 128 * d if d >= 1 else 0
                return d, lo, 512 - lo

            # ledger: ns of TensorE work vs ACT work emitted so far
            led = {"T": 0.0, "A": 0.0}
            LEAD = 3600.0

            # fill queue: (deadline_tile_idx, cost_ns, fn)
            from collections import deque

            fill_q = deque()
            QK_COST, O_COST = 1720.0, 880.0

            def drain_one():
                _, cost, fn = fill_q.popleft()
                fn()
                led["T"] += cost

            def drain_forced(i):
                while fill_q and fill_q[0][0] <= i:
                    drain_one()

            def drain_voluntary():
                while fill_q and led["T"] < led["A"] + LEAD:
                    drain_one()

            # upfront: exactly what block (p0,qb0) needs beyond the queue
            qk_chain(0, 0, 0, 0)()  # Q pair0 t 0:512
            qk_chain(0, 1, 0, 0)()  # K pair0 k 0:512
            for tidx in range(4):
                v_chain(tidx)()

            # static fills with deadlines (monotone)
            def bs(p, qb):
                return bstart[blocks.index((p, qb))]

            # Deadlines are the stream index of each chain's FIRST consumer
            # tile (Q h-slab -> first S of its query block; K h-slab -> first
            # S with j in its k range; V t-tile -> its PV tile), so forced
            # drains spread instead of bursting at block starts.
            ddl = []
            ddl.append((bs(0, 1), qk_chain(0, 0, 0, 1)))          # Q q512:1024
            ddl.append((bs(0, 1) + 4, qk_chain(0, 1, 0, 1)))      # K k512:1024
            for tidx in range(4, 8):
                ddl.append((bs(0, 1) + tidx, v_chain(tidx)))
            ddl.append((bs(0, 2), qk_chain(0, 0, 1, 0)))
            ddl.append((bs(0, 2) + 8, qk_chain(0, 1, 1, 0)))
            for tidx in range(8, 12):
                ddl.append((bs(0, 2) + tidx, v_chain(tidx)))
            ddl.append((bs(0, 3), qk_chain(0, 0, 1, 1)))
            ddl.append((bs(0, 3) + 12, qk_chain(0, 1, 1, 1)))
            for tidx in range(12, 16):
                ddl.append((bs(0, 3) + tidx, v_chain(tidx)))
            for p in (1, 2):
                ddl.append((bs(p, 0), qk_chain(p, 0, 0, 0)))
                ddl.append((bs(p, 0), qk_chain(p, 1, 0, 0)))
                ddl.append((bs(p, 1), qk_chain(p, 0, 0, 1)))
                ddl.append((bs(p, 1) + 4, qk_chain(p, 1, 0, 1)))
                ddl.append((bs(p, 2), qk_chain(p, 0, 1, 0)))
                ddl.append((bs(p, 2) + 8, qk_chain(p, 1, 1, 0)))
                ddl.append((bs(p, 3), qk_chain(p, 0, 1, 1)))
                ddl.append((bs(p, 3) + 12, qk_chain(p, 1, 1, 1)))
            # pair 3 (descending walk, qb3 first): K h-slabs staggered by
            # first consumer j; the three late-query Q chains are reserved as
            # pair3's only fill work during its largest blocks.
            ddl.append((bs(3, 3), qk_chain(3, 1, 0, 0)))
            ddl.append((bs(3, 3), qk_chain(3, 0, 1, 1)))          # Q q1536:2048
            ddl.append((bs(3, 3) + 4, qk_chain(3, 1, 0, 1)))
            ddl.append((bs(3, 3) + 8, qk_chain(3, 1, 1, 0)))
            ddl.append((bs(3, 3) + 12, qk_chain(3, 1, 1, 1)))
            ddl.append((bs(3, 2), qk_chain(3, 0, 1, 0)))          # q 1024:1536
            ddl.append((bs(3, 1), qk_chain(3, 0, 0, 1)))          # q  512:1024
            ddl.append((bs(3, 0), qk_chain(3, 0, 0, 0)))          # q    0:512
            ddl.sort(key=lambda e: e[0])
            for dd, fn in ddl:
                fill_q.append((dd, QK_COST, fn))

            # ---- per-tile emitters ----
            tile_pt = {}
            block_ot = {}

            def emit_S(i):
                bi, p, qb, j = tiles[i]
                d, lo, N = tile_geom(qb, j)
                vq = slice(qb * 512 + lo, (qb + 1) * 512)
                jsl = slice(j * 128, (j + 1) * 128)
                st = ps_st.tile([128, 1024], F32, tag="st")
                st3 = st[:].rearrange("p (h q) -> p h q", h=2)
                nc.tensor.matmul(
                    st3[:, 0, lo:512], kt_sb[0:64, p, jsl], qt_sb[0:64, p, vq]
                )
                nc.tensor.matmul(
                    st3[:, 1, lo:512], kt_sb[64:128, p, jsl], qt_sb[64:128, p, vq]
                )
                led["T"] += 0.78 * N
                return st3

            def emit_E(i, st3):
                bi, p, qb, j = tiles[i]
                d, lo, N = tile_geom(qb, j)
                pt = sb_pt.tile([128, 1024], BF16, tag="pt")
                pt3 = pt[:].rearrange("p (h q) -> p h q", h=2)
                nc.scalar.activation(
                    pt3[:, :, lo:512], st3[:, :, lo:512], AF.Exp, scale=SCALE
                )
                if d >= 0:
                    nc.vector.tensor_mul(
                        pt3[:, :, lo : lo + 128],
                        pt3[:, :, lo : lo + 128],
                        mask_sb[:],
                    )
                led["A"] += 240.0 + 1.71 * N
                tile_pt[i] = pt3

            def emit_PV(i):
                bi, p, qb, j = tiles[i]
                d, lo, N = tile_geom(qb, j)
                pt3 = tile_pt.pop(i)
                first = j == 0
                last = i + 1 >= NT or tiles[i + 1][0] != bi
                if first:
                    ot0 = ps_ot.tile([128, 512], F32, tag="ot")
                    ot1 = ps_ot.tile([128, 512], F32, tag="ot")
                    block_ot[bi] = (ot0, ot1)
                ot0, ot1 = block_ot[bi]
                h0 = 2 * p
                nc.tensor.matmul(
                    ot0[0:65, lo:512],
                    vaug_sb[:, j, h0 * 65 : (h0 + 1) * 65],
                    pt3[:, 0, lo:512],
                    start=first,
                    stop=last,
                )
                nc.tensor.matmul(
                    ot1[0:65, lo:512],
                    vaug_sb[:, j, (h0 + 1) * 65 : (h0 + 2) * 65],
                    pt3[:, 1, lo:512],
                    start=first,
                    stop=last,
                )
                led["T"] += 0.85 * N
                return last

            def part1(bi, tail=False):
                # evacuate O^T and the l row promptly to release the ot banks
                ot0, ot1 = block_ot.pop(bi)
                otus, lrows = [], []
                for hh, ot in ((0, ot0), (1, ot1)):
                    otu = sb_otu.tile([64, 512], BF16, tag="otu")
                    if tail and hh == 1:
                        nc.scalar.copy(otu[:], ot[0:64, :])
                    else:
                        nc.vector.tensor_copy(otu[:], ot[0:64, :])
                    lrow = sb_lr.tile([1, 512], F32, tag="lrow")
                    if hh == 0:
                        nc.scalar.copy(lrow[:], ot[64:65, :])
                    else:
                        nc.vector.tensor_copy(lrow[:], ot[64:65, :])
                    otus.append(otu)
                    lrows.append(lrow)
                return otus, lrows

            def make_part2(bi, otus, lrows, mul_eng=None):
                p, qb = blocks[bi]
                qsl = slice(qb * 512, (qb + 1) * 512)
                eng = mul_eng if mul_eng is not None else nc.vector

                def fn():
                    # both recips first so the muls (which wait on the gpsimd
                    # broadcasts) never head-of-line block the DVE queue;
                    # h1 first: its partition-crossing DMA overlaps h0's mul
                    rbrs = {}
                    for hh in (1, 0):
                        rec = sb_lr.tile([1, 512], F32, tag="rec")
                        nc.vector.reciprocal_approx_fast(rec[:], lrows[hh][:])
                        rbr = sb_rbr.tile([64, 512], F32, tag="rbr")
                        nc.gpsimd.partition_broadcast(rbr[:], rec[0:1, :])
                        rbrs[hh] = rbr
                    sc = sb_sc.tile([64, 512], BF16, tag="sc")
                    eng.tensor_mul(sc[:], otus[1][:], rbrs[1][:])
                    nc.sync.dma_start(otn_sb[64:128, p, qsl], sc[:])
                    eng.tensor_mul(otn_sb[0:64, p, qsl], otus[0][:], rbrs[0][:])

                return fn

            # ---- main pipelined loop ----
            part2_q = []  # (due_iteration, fn, block_idx)

            def maybe_part2(k):
                while part2_q and k >= part2_q[0][0]:
                    due, fn, bi = part2_q.pop(0)
                    fn()
                    p, qb = blocks[bi]
                    if p == 3 and qb > 0:
                        for tb in range(4 * qb, 4 * qb + 4):
                            for half in (0, 1):
                                fill_q.append((NT + 1, O_COST, o_chain(tb, half)))

            def sched_part2(k, bi_prev):
                # Normal: emit the deferred normalize at the 2nd tile of the
                # next block.  For pair-transition blocks (p,3) p<3 the otn
                # output has no consumer until pair 3, and the next block
                # (qb0) is all-diagonal (its mask-muls would sit behind the
                # part2 DVE work): defer clear into (p+1,qb2), whose first
                # diagonal tile is 8 tiles in.  NOTE: gpsimd cannot take the
                # muls -- switching gpsimd between tensor ops and
                # partition_broadcast forces a ~7us library reload.
                p, qb = blocks[bi_prev]
                if qb == 3 and bi_prev + 1 < len(blocks) and blocks[bi_prev + 1][1] == 0:
                    due = bstart[bi_prev + 3] + 2
                else:
                    due = k + 1
                otus, lrows = part1(bi_prev)
                entry = (due, make_part2(bi_prev, otus, lrows), bi_prev)
                part2_q.append(entry)
                part2_q.sort(key=lambda e: e[0])

            for k in range(NT):
                drain_forced(k)
                st3 = emit_S(k)
                emit_E(k, st3)
                maybe_part2(k)
                drain_voluntary()
                if k > 0:
                    last = emit_PV(k - 1)
                    if last:
                        sched_part2(k, tiles[k - 1][0])

            emit_PV(NT - 1)
            bi_last = tiles[NT - 1][0]
            otus, lrows = part1(bi_last, tail=True)
            maybe_part2(NT)  # flush any deferred part2s
            make_part2(bi_last, otus, lrows)()

            # post-stream: remaining out-proj chains (qb1's, then qb0's)
            # rotate over all three PSUM pools so 6 accumulators pipeline;
            # evacuations alternate DVE/ACT.
            TAIL_MODE[0] = True
            while fill_q:
                drain_one()
            n = 0
            for tb in range(4):
                for half in (0, 1):
                    o_chain(tb, half, evac=n % 2)()
                    n += 1

    nc.compile()
    return nc


def _prep_core_inputs(X, W_q, W_k, W_v, W_o, mask_host, c):
    b, half = c // 2, c % 2
    ch = slice(half * HALF, (half + 1) * HALF)
    bf = ml_dtypes.bfloat16
    xtp = np.ascontiguousarray(
        X[b].T.reshape(KB, 128, T).transpose(1, 0, 2)
    )  # [128, KB, T]
    xt = np.ascontiguousarray(
        np.stack([xtp[:, :, q * 512 : (q + 1) * 512] for q in range(4)])
    ).astype(bf)  # [4, 128, KB, 512]
    wqt = np.ascontiguousarray(
        W_q[ch, :].T.reshape(KB, 128, HALF).transpose(1, 0, 2)
    ).astype(bf)
    wkt = np.ascontiguousarray(
        W_k[ch, :].T.reshape(KB, 128, HALF).transpose(1, 0, 2)
    ).astype(bf)
    wvt = np.ascontiguousarray(
        W_v[ch, :].T.reshape(KB, 128, HALF).transpose(1, 0, 2)
    ).astype(bf)
    wot = np.ascontiguousarray(
        W_o[:, ch].T.reshape(4, 128, D).transpose(1, 0, 2)
    ).astype(bf)
    return {
        "xt": xt, "wqt": wqt, "wkt": wkt, "wvt": wvt, "wot": wot,
        "mask": mask_host,
    }


def _make_mask():
    kp = np.arange(128)[:, None]
    qf = np.arange(128)[None, :]
    keep = (qf >= kp).astype(np.float32)
    m = np.zeros((128, 2, 128), np.float32)
    m[:, 0, :] = keep
    m[:, 1, :] = keep
    return m.astype(ml_dtypes.bfloat16)


def kernel(X, W_q, W_k, W_v, W_o):
    global _PROG
    X = np.asarray(X, dtype=np.float32)
    W_q = np.asarray(W_q, dtype=np.float32)
    W_k = np.asarray(W_k, dtype=np.float32)
    W_v = np.asarray(W_v, dtype=np.float32)
    W_o = np.asarray(W_o, dtype=np.float32)

    if _PROG is None:
        _PROG = _build_program()
    nc = _PROG

    mask_host = _make_mask()
    in_maps = [
        _prep_core_inputs(X, W_q, W_k, W_v, W_o, mask_host, c)
        for c in range(N_CORES)
    ]
    res = bass_utils.run_bass_kernel_spmd(nc, in_maps, core_ids=list(range(N_CORES)))

    out = np.empty((B, T, D), np.float32)
    for b in range(B):
        p0 = res.results[2 * b]["out"].reshape(T, D).astype(np.float32)
        p1 = res.results[2 * b + 1]["out"].reshape(T, D).astype(np.float32)
        out[b] = p0 + p1
    return out
